# revision 8
# baseline (speedup 1.0000x reference)
"""Trainium2 Bass kernel for nn_Aligner: GRU + location-sensitive attention scan.

Data-parallel over batch B=64 across 8 NeuronCores (8 examples/core), zero
cross-core communication. All weights SBUF-resident (bf16); the T=800 scan
runs fully on-chip in a tc.For_i loop.

Structure per step (per core, 8 examples):
- prev = E^T alpha via masked-column matmuls (lhsT [128,8] with alpha_b in
  col b only) -> [8,512] PSUM; transposed to feature-major via PE transpose.
- GRU pre-activations weight-STATIONARY: lhsT = 128x128 bf16 weight blocks,
  rhs = feature-major [128,8] activations; PSUM [128,256] = r|z|ni|nh slices;
  frame projections + all biases injected via identity matmul from a
  host-precomputed per-t table. sigmoid via tanh (0.5 folded into weights).
- score = conv(align) + q + key_proj in [L=128, (b,s)] layout: one K=39
  matmul (rows 0:8 one-hot inject q, rows 8:39 conv taps over a DMA-built
  unfold), + identity-matmul kp accumulate; ACT tanh (bias=conv_b, agg_w
  sign folded into all score weights); energy reduce via |agg_w| masked
  columns -> [8,512] directly. softmax/transition all via tanh/exp only.
"""
import sys
sys.path.insert(0, "/opt/trn_rl_repo")
import numpy as np
import ml_dtypes

BF16 = ml_dtypes.bfloat16
N_CORES = 8
import os
B, S, I_, H_, C_, L_, K_, T_ = 64, 512, 512, 80, 1024, 128, 31, 800
T_ = int(os.environ.get("ALIGNER_T", T_))
BL = B // N_CORES
PAD = K_ // 2
SPAD = S + 2 * PAD
_cache = {}


def _build_program():
    import concourse.bass as bass
    import concourse.mybir as mybir
    import concourse.bacc as bacc
    import concourse.tile as tile
    f32 = mybir.dt.float32
    bf16 = mybir.dt.bfloat16
    AF = mybir.ActivationFunctionType
    ALU = mybir.AluOpType

    nc = bacc.Bacc("TRN2", target_bir_lowering=False, debug=False,
                   num_devices=N_CORES)

    d_E = nc.dram_tensor("eT", [128, BL * 4 * 512], bf16, kind="ExternalInput")
    d_kp = nc.dram_tensor("kp", [128, BL * 512], bf16, kind="ExternalInput")
    d_wgi = nc.dram_tensor("wgi", [128, 4 * 24 * 128], bf16, kind="ExternalInput")
    d_wgh = nc.dram_tensor("wgh", [128, 8 * 24 * 128], bf16, kind="ExternalInput")
    d_wt1 = nc.dram_tensor("wt1", [128, 12 * 8 * 128], bf16, kind="ExternalInput")
    d_wq = nc.dram_tensor("wq", [128, 8 * 128], bf16, kind="ExternalInput")
    d_wt2 = nc.dram_tensor("wt2", [128, 16], bf16, kind="ExternalInput")
    d_gif = nc.dram_tensor("gif", [T_, 128, 256], bf16, kind="ExternalInput")
    d_t1f = nc.dram_tensor("t1f", [T_, 128, 64], bf16, kind="ExternalInput")
    d_cw = nc.dram_tensor("cw", [31, 128], bf16, kind="ExternalInput")
    d_cb = nc.dram_tensor("cb", [128, 1], f32, kind="ExternalInput")
    d_aggm = nc.dram_tensor("aggm", [128, 64], bf16, kind="ExternalInput")
    d_i128b = nc.dram_tensor("i128b", [128, 128], bf16, kind="ExternalInput")
    d_i128f = nc.dram_tensor("i128f", [128, 128], f32, kind="ExternalInput")
    d_a0 = nc.dram_tensor("a0", [BL, 512], f32, kind="ExternalInput")
    d_am0 = nc.dram_tensor("am0", [128, 256], bf16, kind="ExternalInput")
    d_ap0 = nc.dram_tensor("ap0", [BL, SPAD], bf16, kind="ExternalInput")
    d_u0 = nc.dram_tensor("u0", [39, BL * 512], bf16, kind="ExternalInput")
    d_magic = nc.dram_tensor("maskadd", [BL, 512], f32, kind="ExternalInput")
    d_t2b = nc.dram_tensor("t2bd", [BL, 1], f32, kind="ExternalInput")
    d_out = nc.dram_tensor("out", [BL, T_, 512], f32, kind="ExternalOutput")

    with tile.TileContext(nc) as tc:
        with (
            tc.tile_pool(name="const", bufs=1) as cpool,
            tc.tile_pool(name="state", bufs=1) as spool,
            tc.tile_pool(name="wbig", bufs=1) as wbig,
            tc.tile_pool(name="wpre", bufs=2) as wpre,
            tc.tile_pool(name="work", bufs=1) as wpool,
            tc.tile_pool(name="ps_big", bufs=1, space="PSUM") as ps_big,
            tc.tile_pool(name="ps_gru", bufs=1, space="PSUM") as ps_gru,
            tc.tile_pool(name="ps_sm", bufs=4, space="PSUM") as ps_sm,
            tc.tile_pool(name="dram", bufs=1, space="DRAM") as dpool,
        ):
            E = cpool.tile([128, BL * 4 * 512], bf16, tag="E")
            kp = cpool.tile([128, BL * 512], bf16, tag="kp")
            wgi = cpool.tile([128, 4 * 24 * 128], bf16, tag="wgi")
            wgh = cpool.tile([128, 8 * 24 * 128], bf16, tag="wgh")
            wt1 = cpool.tile([128, 12 * 8 * 128], bf16, tag="wt1")
            wq = cpool.tile([128, 8 * 128], bf16, tag="wq")
            wt2 = cpool.tile([128, 16], bf16, tag="wt2")
            cb = cpool.tile([128, 1], f32, tag="cb")
            aggm = cpool.tile([128, 64], bf16, tag="aggm")
            i128b = cpool.tile([128, 128], bf16, tag="i128b")
            i128f = cpool.tile([128, 128], f32, tag="i128f")
            maskadd = cpool.tile([BL, 512], f32, tag="maskadd")
            t2bd = cpool.tile([BL, 1], f32, tag="t2bd")
            for dst, src in [(E, d_E), (kp, d_kp), (wgi, d_wgi), (wgh, d_wgh),
                             (wt1, d_wt1), (wq, d_wq), (wt2, d_wt2),
                             (cb, d_cb), (aggm, d_aggm), (i128b, d_i128b),
                             (i128f, d_i128f), (maskadd, d_magic),
                             (t2bd, d_t2b)]:
                nc.sync.dma_start(dst[:], src.ap())

            hT = spool.tile([128, 64], bf16, tag="hT")
            hF = spool.tile([128, 64], f32, tag="hF")
            alpha = spool.tile([BL, 512], f32, tag="alpha")
            AM = spool.tile([128, 256], bf16, tag="AM")
            U = spool.tile([39, BL * 512], bf16, tag="U")
            convq = spool.tile([39, 128], bf16, tag="convq")
            apad = dpool.tile([BL, SPAD], bf16, tag="apad")

            nc.vector.memset(hT[:], 0)
            nc.vector.memset(hF[:], 0)
            nc.sync.dma_start(alpha[:], d_a0.ap())
            nc.sync.dma_start(AM[:], d_am0.ap())
            nc.sync.dma_start(U[:], d_u0.ap())
            nc.sync.dma_start(apad[:], d_ap0.ap())
            nc.sync.dma_start(convq[8:39, :], d_cw.ap())

            apad_ap = apad[:].opt()
            unfold_src = bass.AP(tensor=apad_ap.tensor, offset=apad_ap.offset,
                                 ap=[[1, 31], [SPAD, BL], [1, 512]])

            with tc.For_i(0, T_, 1) as iv:
                gif = wpre.tile([128, 256], bf16, tag="gif")
                t1f = wpre.tile([128, 64], bf16, tag="t1f")
                nc.sync.dma_start(gif[:], d_gif.ap()[bass.ds(iv, 1)])
                nc.sync.dma_start(t1f[:], d_t1f.ap()[bass.ds(iv, 1)])
                nc.sync.dma_start(U[8:39, :], unfold_src)

                # ---- prev = E^T alpha ----
                prev_ps = ps_sm.tile([BL, 512], f32, tag="sm")
                for sc in range(4):
                    for b in range(BL):
                        nc.tensor.matmul(
                            prev_ps[:],
                            AM[:, sc * 64 + b * 8:sc * 64 + (b + 1) * 8],
                            E[:, (b * 4 + sc) * 512:(b * 4 + sc + 1) * 512],
                            start=(sc == 0 and b == 0),
                            stop=(sc == 3 and b == BL - 1))
                prevS = wpool.tile([BL, 512], f32, tag="prevS")
                nc.vector.tensor_copy(prevS[:], prev_ps[:])
                xT = wpool.tile([128, 32], bf16, tag="xT")
                for ic in range(4):
                    pT = ps_sm.tile([128, BL], f32, tag="sm")
                    nc.tensor.transpose(pT[:], prevS[:, ic * 128:(ic + 1) * 128],
                                        i128f[:BL, :BL])
                    nc.vector.tensor_copy(xT[:, ic * 8:(ic + 1) * 8], pT[:])

                # ---- GRU pre-activations ----
                g_ps = ps_gru.tile([128, 256], f32, tag="g")
                nc.tensor.matmul(g_ps[:], i128b[:], gif[:], start=True, stop=False)
                for kc in range(8):
                    for m in range(24):
                        gate, j = divmod(m, 8)
                        col = gate * 64 + j * 8 if gate < 2 else 192 + j * 8
                        nc.tensor.matmul(
                            g_ps[:, col:col + 8],
                            wgh[:, (kc * 24 + m) * 128:(kc * 24 + m + 1) * 128],
                            hT[:, kc * 8:(kc + 1) * 8], start=False, stop=False)
                for kc in range(4):
                    for m in range(24):
                        gate, j = divmod(m, 8)
                        col = gate * 64 + j * 8 if gate < 2 else 128 + j * 8
                        nc.tensor.matmul(
                            g_ps[:, col:col + 8],
                            wgi[:, (kc * 24 + m) * 128:(kc * 24 + m + 1) * 128],
                            xT[:, kc * 8:(kc + 1) * 8],
                            start=False, stop=(kc == 3 and m == 23))

                # ---- GRU elementwise ----
                tr_ = wpool.tile([128, 64], f32, tag="tr")
                tz_ = wpool.tile([128, 64], f32, tag="tz")
                nc.scalar.activation(tr_[:], g_ps[:, 0:64], AF.Tanh)
                nc.scalar.activation(tz_[:], g_ps[:, 64:128], AF.Tanh)
                u_ = wpool.tile([128, 64], f32, tag="u")
                v_ = wpool.tile([128, 64], f32, tag="v")
                w_ = wpool.tile([128, 64], f32, tag="w")
                Psb = wpool.tile([128, 64], f32, tag="Psb")
                nc.vector.tensor_copy(Psb[:], g_ps[:, 192:256])
                nc.vector.tensor_add(u_[:], g_ps[:, 128:192], Psb[:])
                nc.vector.tensor_mul(v_[:], tr_[:], Psb[:])
                nc.vector.tensor_add(w_[:], u_[:], v_[:])
                n_ = wpool.tile([128, 64], f32, tag="n")
                nc.scalar.activation(n_[:], w_[:], AF.Tanh)
                d_t = wpool.tile([128, 64], f32, tag="d")
                e_t = wpool.tile([128, 64], f32, tag="e")
                s_t = wpool.tile([128, 64], f32, tag="s")
                nc.vector.tensor_sub(d_t[:], n_[:], hF[:])
                nc.vector.tensor_mul(e_t[:], tz_[:], d_t[:])
                nc.vector.tensor_add(s_t[:], d_t[:], e_t[:])
                nc.vector.scalar_tensor_tensor(hF[:], s_t[:], -0.5, n_[:],
                                               ALU.mult, ALU.add)
                nc.vector.tensor_copy(hT[:], hF[:])

                # ---- q -> convq rows 0:8 ----
                q_ps = ps_sm.tile([128, BL], f32, tag="sm")
                for kc in range(8):
                    nc.tensor.matmul(q_ps[:], wq[:, kc * 128:(kc + 1) * 128],
                                     hT[:, kc * 8:(kc + 1) * 8],
                                     start=(kc == 0), stop=(kc == 7))
                qf = wpool.tile([128, BL], bf16, tag="qf")
                nc.vector.tensor_copy(qf[:], q_ps[:])
                qT_ps = ps_sm.tile([BL, 128], bf16, tag="sm")
                nc.tensor.transpose(qT_ps[:], qf[:], i128b[:])
                nc.vector.tensor_copy(convq[0:8, :], qT_ps[:])

                # ---- transition agent ----
                t1_ps = ps_sm.tile([128, 64], f32, tag="sm")
                nc.tensor.matmul(t1_ps[:], i128b[:], t1f[:], start=True, stop=False)
                for kc in range(4):
                    for m in range(8):
                        nc.tensor.matmul(
                            t1_ps[:, m * 8:(m + 1) * 8],
                            wt1[:, (kc * 8 + m) * 128:(kc * 8 + m + 1) * 128],
                            xT[:, kc * 8:(kc + 1) * 8], start=False, stop=False)
                for kc in range(8):
                    for m in range(8):
                        nc.tensor.matmul(
                            t1_ps[:, m * 8:(m + 1) * 8],
                            wt1[:, ((4 + kc) * 8 + m) * 128:((4 + kc) * 8 + m + 1) * 128],
                            hT[:, kc * 8:(kc + 1) * 8],
                            start=False, stop=(kc == 7 and m == 7))
                t1t = wpool.tile([128, 64], bf16, tag="t1t")
                nc.scalar.activation(t1t[:], t1_ps[:], AF.Tanh)
                t2_ps = ps_sm.tile([BL, 2], f32, tag="sm")
                for kc in range(8):
                    nc.tensor.matmul(t2_ps[:], t1t[:, kc * 8:(kc + 1) * 8],
                                     wt2[:, kc * 2:(kc + 1) * 2],
                                     start=(kc == 0), stop=(kc == 7))
                t2sb = wpool.tile([BL, 2], f32, tag="t2sb")
                nc.vector.tensor_copy(t2sb[:], t2_ps[:])
                dt2 = wpool.tile([BL, 1], f32, tag="dt2")
                nc.vector.tensor_sub(dt2[:], t2sb[:, 0:1], t2sb[:, 1:2])
                dt3 = wpool.tile([BL, 1], f32, tag="dt3")
                nc.vector.tensor_add(dt3[:], dt2[:], t2bd[:])
                tau = wpool.tile([BL, 1], f32, tag="tau")
                nc.scalar.activation(tau[:], dt3[:], AF.Tanh, scale=0.5)
                stp = wpool.tile([BL, 1], f32, tag="stp")
                nxt = wpool.tile([BL, 1], f32, tag="nxt")
                nc.vector.tensor_scalar(stp[:], tau[:], 0.5, 0.5, ALU.mult, ALU.add)
                nc.vector.tensor_scalar(nxt[:], tau[:], -0.5, 0.5, ALU.mult, ALU.add)

                # ---- score / tanh / energy ----
                en_ps = ps_sm.tile([BL, 512], f32, tag="sm")
                tanh_sb = wbig.tile([128, BL * 512], bf16, tag="tanh")
                for qt in range(4):
                    c0 = qt * 1024
                    sc_ps = ps_big.tile([128, 1024], f32, tag="score")
                    for h2 in range(2):
                        cc = c0 + h2 * 512
                        nc.tensor.matmul(sc_ps[:, h2 * 512:(h2 + 1) * 512],
                                         convq[:], U[:, cc:cc + 512],
                                         start=True, stop=False)
                        nc.tensor.matmul(sc_ps[:, h2 * 512:(h2 + 1) * 512],
                                         i128b[:], kp[:, cc:cc + 512],
                                         start=False, stop=True)
                    nc.scalar.activation(tanh_sb[:, c0:c0 + 1024], sc_ps[:],
                                         AF.Tanh, bias=cb[:])
                    for bb in range(2):
                        b = qt * 2 + bb
                        nc.tensor.matmul(en_ps[:],
                                         aggm[:, b * 8:(b + 1) * 8],
                                         tanh_sb[:, b * 512:(b + 1) * 512],
                                         start=(b == 0), stop=(b == BL - 1))

                # ---- align softmax ----
                en2 = wpool.tile([BL, 512], f32, tag="en2")
                nc.vector.tensor_add(en2[:], en_ps[:], maskadd[:])
                mx = wpool.tile([BL, 1], f32, tag="mx")
                nc.vector.tensor_reduce(mx[:], en2[:], mybir.AxisListType.X,
                                        ALU.max, negate=True)
                align_raw = wpool.tile([BL, 512], f32, tag="align_raw")
                z2 = wpool.tile([BL, 1], f32, tag="z2")
                nc.scalar.activation(align_raw[:], en2[:], AF.Exp, bias=mx[:],
                                     accum_out=z2[:])
                rz2 = wpool.tile([BL, 1], f32, tag="rz2")
                nc.vector.reciprocal(rz2[:], z2[:])
                align_bf = wpool.tile([BL, 512], bf16, tag="align_bf")
                nc.vector.tensor_scalar_mul(align_bf[:], align_raw[:], rz2[:])
                nc.sync.dma_start(apad[:, PAD:PAD + 512], align_bf[:])

                # ---- alpha update ----
                sh = wpool.tile([BL, 512], f32, tag="sh")
                nc.vector.memset(sh[:, 0:1], 0)
                nc.vector.tensor_scalar_mul(sh[:, 1:512], alpha[:, 0:511], nxt[:])
                s2 = wpool.tile([BL, 512], f32, tag="s2")
                nc.vector.scalar_tensor_tensor(s2[:], alpha[:], stp[:], sh[:],
                                               ALU.mult, ALU.add)
                ap2 = wpool.tile([BL, 512], f32, tag="ap2")
                zs = wpool.tile([BL, 1], f32, tag="zs")
                nc.vector.scalar_tensor_tensor(ap2[:], s2[:], 1e-5, align_bf[:],
                                               ALU.add, ALU.mult, accum_out=zs[:])
                rzs = wpool.tile([BL, 1], f32, tag="rzs")
                nc.vector.reciprocal(rzs[:], zs[:])
                nc.vector.tensor_scalar_mul(alpha[:], ap2[:], rzs[:])
                nc.sync.dma_start(d_out.ap()[:, bass.ds(iv, 1)], alpha[:])

                # ---- alpha^T -> masked AM tiles ----
                for sc in range(4):
                    aT_ps = ps_sm.tile([128, BL], f32, tag="sm")
                    nc.tensor.transpose(aT_ps[:], alpha[:, sc * 128:(sc + 1) * 128],
                                        i128f[:BL, :BL])
                    for b in range(BL):
                        nc.vector.tensor_copy(
                            AM[:, sc * 64 + b * 8 + b:sc * 64 + b * 8 + b + 1],
                            aT_ps[:, b:b + 1])
    nc.compile()
    return nc


def _host_prep(inputs):
    enc = np.asarray(inputs["encodings"], np.float32)
    mask = np.asarray(inputs["mask"], np.float32)
    gt = np.asarray(inputs["gt"], np.float32)
    w_ih = np.asarray(inputs["w_ih"], np.float32)
    w_hh = np.asarray(inputs["w_hh"], np.float32)
    b_ih = np.asarray(inputs["b_ih"], np.float32)
    b_hh = np.asarray(inputs["b_hh"], np.float32)
    wq = np.asarray(inputs["wq"], np.float32)
    wk = np.asarray(inputs["wk"], np.float32)
    conv_w = np.asarray(inputs["conv_w"], np.float32)
    conv_b = np.asarray(inputs["conv_b"], np.float32)
    agg_w = np.asarray(inputs["agg_w"], np.float32)
    t1_w = np.asarray(inputs["t1_w"], np.float32)
    t1_b = np.asarray(inputs["t1_b"], np.float32)
    t2_w = np.asarray(inputs["t2_w"], np.float32)
    t2_b = np.asarray(inputs["t2_b"], np.float32)

    sgn = np.sign(agg_w[0]).astype(np.float32)
    sgn[sgn == 0] = 1.0
    aag = np.abs(agg_w[0]).astype(np.float32)

    wih = w_ih.copy(); whh = w_hh.copy()
    wih[:2 * C_] *= 0.5
    whh[:2 * C_] *= 0.5
    whh[2 * C_:] *= 0.5

    def blockize(mat, nkc, nm):  # mat [nkc*128, nm*128] -> [128, nkc*nm*128]
        out = np.zeros((128, nkc * nm * 128), np.float32)
        for kc in range(nkc):
            for m in range(nm):
                out[:, (kc * nm + m) * 128:(kc * nm + m + 1) * 128] = \
                    mat[kc * 128:(kc + 1) * 128, m * 128:(m + 1) * 128]
        return out

    wgi = blockize(wih[:, H_:].T, 4, 24)
    wgh = blockize(whh.T, 8, 24)
    wt1 = blockize(t1_w[:, H_:].T, 12, 8)
    wqb = blockize((wq * sgn[:, None]).T, 8, 1)
    t2T = t2_w.T
    wt2 = np.zeros((128, 16), np.float32)
    for kc in range(8):
        wt2[:, kc * 2:(kc + 1) * 2] = t2T[kc * 128:(kc + 1) * 128]
    t2bd = np.full((BL, 1), float(t2_b[0] - t2_b[1]), np.float32)

    gtf = np.ascontiguousarray(gt[:, :T_]).reshape(B * T_, H_)
    gi_f = (gtf @ wih[:, :H_].T).reshape(B, T_, 3 * C_)
    gi_f[:, :, :C_] += 0.5 * (b_ih[:C_] + b_hh[:C_])
    gi_f[:, :, C_:2 * C_] += 0.5 * (b_ih[C_:2 * C_] + b_hh[C_:2 * C_])
    gi_f[:, :, 2 * C_:] += b_ih[2 * C_:]
    nh_bias = 0.5 * b_hh[2 * C_:]
    t1_f = (gtf @ t1_w[:, :H_].T + t1_b).reshape(B, T_, C_)

    kpv = (enc.reshape(B * S, I_) @ (wk * sgn[:, None]).T).reshape(B, S, L_)

    cw = (conv_w[:, 0, :] * sgn[:, None]).T.astype(np.float32)
    cbv = (conv_b * sgn).astype(np.float32)[:, None]
    aggm = np.zeros((128, 64), np.float32)
    for b in range(BL):
        aggm[:, b * 8 + b] = aag
    i128f = np.eye(128, dtype=np.float32)

    in_maps = []
    for c in range(N_CORES):
        bsl = slice(c * BL, (c + 1) * BL)
        e_c = enc[bsl]
        eT = np.zeros((128, BL * 4 * 512), np.float32)
        for b in range(BL):
            for sc in range(4):
                eT[:, (b * 4 + sc) * 512:(b * 4 + sc + 1) * 512] = \
                    e_c[b, sc * 128:(sc + 1) * 128, :]
        kpc = np.zeros((128, BL * 512), np.float32)
        for b in range(BL):
            kpc[:, b * 512:(b + 1) * 512] = kpv[c * BL + b].T
        g = gi_f[bsl]
        gifc = np.zeros((T_, 128, 256), np.float32)
        for j in range(8):
            for b in range(BL):
                gifc[:, :, 0 + j * 8 + b] = g[b, :, j * 128:(j + 1) * 128]
                gifc[:, :, 64 + j * 8 + b] = g[b, :, C_ + j * 128:C_ + (j + 1) * 128]
                gifc[:, :, 128 + j * 8 + b] = g[b, :, 2 * C_ + j * 128:2 * C_ + (j + 1) * 128]
            gifc[:, :, 192 + j * 8:192 + (j + 1) * 8] = \
                nh_bias[j * 128:(j + 1) * 128, None]
        tf = t1_f[bsl]
        t1fc = np.zeros((T_, 128, 64), np.float32)
        for j in range(8):
            for b in range(BL):
                t1fc[:, :, j * 8 + b] = tf[b, :, j * 128:(j + 1) * 128]
        a0 = np.zeros((BL, 512), np.float32); a0[:, 0] = 1.0
        am0 = np.zeros((128, 256), np.float32)
        for b in range(BL):
            am0[0, b * 8 + b] = 1.0
        ap0 = np.zeros((BL, SPAD), np.float32); ap0[:, PAD] = 1.0
        u0 = np.zeros((39, BL * 512), np.float32)
        for b in range(BL):
            u0[b, b * 512:(b + 1) * 512] = 1.0
        maskadd = (mask[bsl] - 1.0) * 1e30

        in_maps.append(dict(
            eT=eT.astype(BF16), kp=kpc.astype(BF16), wgi=wgi.astype(BF16),
            wgh=wgh.astype(BF16), wt1=wt1.astype(BF16), wq=wqb.astype(BF16),
            wt2=wt2.astype(BF16), gif=gifc.astype(BF16), t1f=t1fc.astype(BF16),
            cw=cw.astype(BF16), cb=cbv, aggm=aggm.astype(BF16),
            i128b=i128f.astype(BF16), i128f=i128f, a0=a0,
            am0=am0.astype(BF16), ap0=ap0.astype(BF16), u0=u0.astype(BF16),
            maskadd=maskadd.astype(np.float32), t2bd=t2bd,
        ))
    return in_maps


class _Runner:
    def __init__(self, nc, n_cores):
        import jax
        from jax.sharding import Mesh, PartitionSpec
        from jax.experimental.shard_map import shard_map
        from concourse.bass2jax import (_bass_exec_p, install_neuronx_cc_hook,
                                        partition_id_tensor)
        import concourse.mybir as mybir
        install_neuronx_cc_hook()
        self.jax = jax
        self.n_cores = n_cores
        pname = nc.partition_id_tensor.name if nc.partition_id_tensor else None
        in_names, out_names, out_avals, zero_outs = [], [], [], []
        for alloc in nc.m.functions[0].allocations:
            if not isinstance(alloc, mybir.MemoryLocationSet):
                continue
            name = alloc.memorylocations[0].name
            if alloc.kind == "ExternalInput":
                if name != pname:
                    in_names.append(name)
            elif alloc.kind == "ExternalOutput":
                shape = tuple(alloc.tensor_shape)
                dtype = mybir.dt.np(alloc.dtype)
                out_avals.append(jax.core.ShapedArray(shape, dtype))
                out_names.append(name)
                zero_outs.append(np.zeros(shape, dtype))
        self.in_names, self.out_names = in_names, out_names
        self.out_avals, self.zero_outs = out_avals, zero_outs
        n_params, n_outs = len(in_names), len(out_avals)
        all_in = in_names + out_names + ([pname] if pname else [])

        def _body(*args):
            operands = list(args)
            if pname is not None:
                operands.append(partition_id_tensor())
            outs = _bass_exec_p.bind(
                *operands, out_avals=tuple(out_avals), in_names=tuple(all_in),
                out_names=tuple(out_names), lowering_input_output_aliases=(),
                sim_require_finite=False, sim_require_nnan=False, nc=nc)
            return tuple(outs)

        devices = jax.devices()[:n_cores]
        mesh = Mesh(np.asarray(devices), ("core",))
        in_specs = (PartitionSpec("core"),) * (n_params + n_outs)
        out_specs = (PartitionSpec("core"),) * n_outs
        self.fn = jax.jit(
            shard_map(_body, mesh=mesh, in_specs=in_specs,
                      out_specs=out_specs, check_rep=False),
            donate_argnums=tuple(range(n_params, n_params + n_outs)),
            keep_unused=True)

    def put(self, in_maps):
        jax = self.jax
        self.dev_in = [jax.device_put(
            np.concatenate([np.asarray(m[n]) for m in in_maps], axis=0))
            for n in self.in_names]
        jax.block_until_ready(self.dev_in)

    def run_cached(self):
        jax = self.jax
        concat_zeros = [np.zeros((self.n_cores * z.shape[0], *z.shape[1:]), z.dtype)
                        for z in self.zero_outs]
        outs = self.fn(*self.dev_in, *concat_zeros)
        jax.block_until_ready(outs)
        return outs

    def run(self, in_maps):
        jax = self.jax
        concat_in = [np.concatenate([np.asarray(m[n]) for m in in_maps], axis=0)
                     for n in self.in_names]
        concat_zeros = [np.zeros((self.n_cores * z.shape[0], *z.shape[1:]), z.dtype)
                        for z in self.zero_outs]
        outs = self.fn(*concat_in, *concat_zeros)
        jax.block_until_ready(outs)
        return [
            {n: np.asarray(outs[i]).reshape(self.n_cores, *self.out_avals[i].shape)[c]
             for i, n in enumerate(self.out_names)}
            for c in range(self.n_cores)
        ]


def _get_runner():
    if "runner" not in _cache:
        nc = _build_program()
        _cache["runner"] = _Runner(nc, N_CORES)
    return _cache["runner"]


def kernel(**inputs):
    runner = _get_runner()
    in_maps = _host_prep(inputs)
    res = runner.run(in_maps)
    return np.concatenate([res[c]["out"] for c in range(N_CORES)], axis=0)
